# revision 1
# baseline (speedup 1.0000x reference)
"""Trainium2 Bass kernel for nn_Decoder (seq2seq BiLSTM encoder + LSTM decoder).

Strategy (8 NeuronCores, SPMD):
  - Gate/hidden-dim model parallelism for both recurrences:
      encoder: fwd chain on one physical quad (4 cores), bwd chain on the
      other; each core owns a 128-row hidden slice (all 4 gates).
      decoder: all 8 cores, each owns a 128-row slice of the 1024 hidden.
  - Per-step h exchange via remote_dma_broadcast (SBUF->SBUF, XOR-relative
    dests).  Receiver slot j holds the slice of the core at phys XOR j; the
    host permutes weight K-chunks per core to match (slot map discovered
    once by a probe kernel).
  - Input-to-hidden projections precomputed for all timesteps as batched
    matmuls (weight-stationary bf16 / FWL).
  - Embedding lookups via dma_gather(transpose=True) directly into the
    transposed [feature, token] layout.
  - Matmuls bf16 -> fp32 PSUM; c state fp32; h carried bf16
    (validated: global rel err ~3e-3 vs fp32 reference).
"""

import sys
import numpy as np
import ml_dtypes

for _p in ("/opt/trn_rl_repo",):
    if _p not in sys.path:
        sys.path.insert(0, _p)

import concourse.bass as bass
import concourse.bacc as bacc
import concourse.mybir as mybir
from concourse.bass import AP
from concourse.bass_utils import run_bass_kernel_spmd

BF16 = mybir.dt.bfloat16
F32 = mybir.dt.float32
NP_BF16 = ml_dtypes.bfloat16

E = 512
V = 32000
B = 32
S = 128
T = 128
HD = 2 * E
NC = 8
AF = mybir.ActivationFunctionType

# gate col order within each 128-block: [i, f, o, g]; pytorch rows: i,f,g,o
_GATE_BLOCK = {0: 0, 1: 1, 2: 3, 3: 2}


class Ctr:
    __slots__ = ("v",)
    def __init__(self):
        self.v = 0
    def add(self, n):
        self.v += n
        return self.v


def _build_probe():
    nc = bacc.Bacc(None, target_bir_lowering=False, num_devices=NC)
    myid = nc.dram_tensor("myid", [128, 32], F32, kind="ExternalInput")
    out = nc.dram_tensor("out", [128, 8 * 32], F32, kind="ExternalOutput")
    recv_sem = nc.alloc_semaphore("recv_sem")
    local_sem = nc.alloc_semaphore("local_sem")
    prep_sem = nc.alloc_semaphore("prep_sem")
    dma_sem = nc.alloc_semaphore("dma_sem")
    own = nc.alloc_sbuf_tensor("own", [128, 32], F32).ap()
    recv = nc.alloc_sbuf_tensor("recv", [128, 8 * 32], F32).ap()
    nc.gpsimd.memset(recv[:, :], -1.0)
    nc.sync.dma_start(out=own[:, :], in_=myid[:]).then_inc(dma_sem, 16)
    nc.all_core_barrier()
    nc.gpsimd.wait_ge(dma_sem, 16)
    nc.vector.tensor_copy(recv[:, 0:32], own[:, :]).then_inc(prep_sem, 1)
    for j in range(1, 8):
        rdests = [None] * 8
        rdests[j] = (0, j)
        nc.gpsimd.remote_dma_broadcast(
            out_ap=recv[:, j * 32:(j + 1) * 32], in_ap=own[:, :],
            remote_sem=recv_sem, local_sem=local_sem, rdests=rdests,
        ).then_inc(prep_sem, 1)
    nc.gpsimd.wait_ge(prep_sem, 8)
    nc.gpsimd.trigger_dma(count=7)
    nc.sync.wait_ge(recv_sem, 14)
    nc.sync.dma_start(out=out[:], in_=recv[:]).then_inc(dma_sem, 16)
    nc.sync.wait_ge(dma_sem, 32)
    nc.compile()
    return nc


_SLOT_MAP = None


def get_slot_map():
    """slot_map[r][j] = logical core whose broadcast lands in slot j on core r."""
    global _SLOT_MAP
    if _SLOT_MAP is not None:
        return _SLOT_MAP
    nc = _build_probe()
    in_maps = [{"myid": np.full((128, 32), float(c), np.float32)} for c in range(NC)]
    res = run_bass_kernel_spmd(nc, in_maps, core_ids=list(range(NC)))
    sm = np.zeros((NC, 8), np.int64)
    for r in range(NC):
        o = res.results[r]["out"]
        for j in range(8):
            v = o[:, j * 32:(j + 1) * 32]
            assert (v == v[0, 0]).all(), f"probe: core {r} slot {j} non-uniform"
            sm[r, j] = int(v[0, 0])
    assert (sm[:, 0] == np.arange(NC)).all(), sm
    for r in range(NC):
        assert sorted(sm[r]) == list(range(8)), sm[r]
        for j in range(8):
            assert sm[sm[r, j], j] == r, (r, j)
    _SLOT_MAP = sm
    return sm


# ---------------------------------------------------------------------------
# kernel builder
# ---------------------------------------------------------------------------

def build_kernel(n_s=S, n_t=T):
    assert (B * n_s) % 512 == 0 and (B * n_t) % 512 == 0
    nc = bacc.Bacc(None, target_bir_lowering=False, num_devices=NC,
                   dynamic_dma_scratch_size=32768)
    ne, ntk = B * n_s, B * n_t

    # ---------------- I/O ----------------
    enc_tab = nc.dram_tensor("enc_tab", [V, E], BF16, kind="ExternalInput")
    dec_tab = nc.dram_tensor("dec_tab", [V, HD], BF16, kind="ExternalInput")
    enc_idx_d = nc.dram_tensor("enc_idx", [128, ne // 16], mybir.dt.int16, kind="ExternalInput")
    dec_idx_d = nc.dram_tensor("dec_idx", [128, ntk // 16], mybir.dt.int16, kind="ExternalInput")
    enc_wT_d = nc.dram_tensor("enc_wT", [128, 4 * 512], BF16, kind="ExternalInput")
    enc_uT_d = nc.dram_tensor("enc_uT", [128, 4 * 512], BF16, kind="ExternalInput")
    enc_b_d = nc.dram_tensor("enc_b", [128, 4], F32, kind="ExternalInput")
    dec_w1T_d = nc.dram_tensor("dec_w1T", [128, 8 * 512], BF16, kind="ExternalInput")
    dec_w2T_d = nc.dram_tensor("dec_w2T", [128, 8 * 512], BF16, kind="ExternalInput")
    dec_uT_d = nc.dram_tensor("dec_uT", [128, 8 * 512], BF16, kind="ExternalInput")
    dec_b_d = nc.dram_tensor("dec_b", [128, 4], F32, kind="ExternalInput")
    out_d = nc.dram_tensor("out", [128, n_t * 32], F32, kind="ExternalOutput")

    # ---------------- SBUF ----------------
    sb = nc.alloc_sbuf_tensor
    enc_wT = sb("enc_wT_s", [128, 4 * 512], BF16).ap()
    enc_uT = sb("enc_uT_s", [128, 4 * 512], BF16).ap()
    enc_b = sb("enc_b_s", [128, 4], F32).ap()
    dec_w1T = sb("dec_w1T_s", [128, 8 * 512], BF16).ap()
    dec_w2T = sb("dec_w2T_s", [128, 8 * 512], BF16).ap()
    dec_uT = sb("dec_uT_s", [128, 8 * 512], BF16).ap()
    dec_b = sb("dec_b_s", [128, 4], F32).ap()
    enc_idx = sb("enc_idx_s", [128, ne // 16], mybir.dt.int16).ap()
    dec_idx = sb("dec_idx_s", [128, ntk // 16], mybir.dt.int16).ap()
    dembT_h = sb("dembT", [128, 8 * ntk], BF16)    # [128][8][ntk]
    # encoder embT aliases the first 4*ne cols of dembT (dead before dec gather)
    if 4 * ne <= 8 * ntk:
        embT_h = dembT_h
        emb_pstride = 8 * ntk
    else:
        embT_h = sb("embT", [128, 4 * ne], BF16)
        emb_pstride = 4 * ne
    ig_e_h = sb("ig_e", [128, n_s * 128], BF16)    # col t*128 + m*32 + b
    ig_d_h = sb("ig_d", [128, n_t * 128], BF16)
    ig_e = ig_e_h.ap()
    ig_d = ig_d_h.ap()
    hT_e = [sb(f"hT_e{p}", [128, 4 * 32], BF16).ap() for p in range(2)]
    hT_d = [sb(f"hT_d{p}", [128, 8 * 32], BF16).ap() for p in range(2)]
    c_e = [sb(f"c_e{p}", [128, 32], F32).ap() for p in range(2)]
    c_d = [sb(f"c_d{p}", [128, 32], F32).ap() for p in range(2)]
    pre = sb("pre", [128, 128], F32).ap()
    act = sb("act", [128, 128], F32).ap()
    tc = sb("tc", [128, 32], F32).ap()
    m1 = sb("m1", [128, 32], F32).ap()
    m2 = sb("m2", [128, 32], F32).ap()
    ctx_g_h = sb("ctx_g", [128, 128], F32)
    ctx_g = ctx_g_h.ap()
    outT = sb("outT", [128, n_t * 32], F32).ap()

    pb = [nc.alloc_psum_tensor(f"pb{i}", [128, 512], F32).ap() for i in range(4)]

    # ---------------- semaphores ----------------
    sem = nc.alloc_semaphore
    s_dma = sem("s_dma");   c_dma = Ctr()
    s_gat_e = sem("s_gat_e")
    s_gat_d = sem("s_gat_d")
    s_relu = sem("s_relu"); c_relu = Ctr()
    s_pe = sem("s_pe");     c_pe = Ctr()
    s_evac = sem("s_evac"); c_evac = Ctr()
    s_add = sem("s_add");   c_add = Ctr()
    s_sig = sem("s_sig");   c_sig = Ctr()
    s_cup = sem("s_cup");   c_cup = Ctr()
    s_tc = sem("s_tc");     c_tc = Ctr()
    s_cast = sem("s_cast"); c_cast = Ctr()
    s_prep = sem("s_prep"); c_prep = Ctr()
    # per-slot, per-buffer-parity receive sems (shared by enc/dec phases);
    # thresholds are taken from emission-time counters, which makes the
    # cumulative waits race-free (see design notes).
    s_recv = {(j, p): sem(f"s_recv_{j}_{p}") for j in range(1, 8) for p in range(2)}
    recv_cnt = {k: 0 for k in s_recv}
    s_loc = [sem("s_loc_0"), sem("s_loc_1")]
    loc_cnt = [0, 0]
    s_misc = sem("s_misc"); c_misc = Ctr()
    s_dvef = sem("s_dvef"); c_dvef = Ctr()

    def dma(dst, src):
        nc.sync.dma_start(out=dst, in_=src).then_inc(s_dma, 16)
        c_dma.add(16)

    # ============ phase L: loads + init ============
    dma(enc_wT[:, :], enc_wT_d[:])
    dma(enc_uT[:, :], enc_uT_d[:])
    dma(enc_b[:, :], enc_b_d[:])
    dma(dec_w1T[:, :], dec_w1T_d[:])
    dma(dec_w2T[:, :], dec_w2T_d[:])
    dma(dec_uT[:, :], dec_uT_d[:])
    dma(dec_b[:, :], dec_b_d[:])
    dma(enc_idx[:, :], enc_idx_d[:])
    dma(dec_idx[:, :], dec_idx_d[:])
    loads_done = c_dma.v

    nc.gpsimd.memset(hT_e[0][:, :], 0.0).then_inc(s_misc, 1); c_misc.add(1)
    nc.gpsimd.memset(c_e[0][:, :], 0.0).then_inc(s_misc, 1); c_misc.add(1)
    nc.gpsimd.memset(c_d[0][:, :], 0.0).then_inc(s_misc, 1); c_misc.add(1)
    memsets_done = c_misc.v

    nc.all_core_barrier()

    # ============ phase G: gathers ============
    nc.gpsimd.wait_ge(s_dma, loads_done)
    n_gchunk_e = ne // 512
    for c in range(n_gchunk_e):
        embT_ap = AP(embT_h, c * 4 * 512, [[emb_pstride, 128], [512, 4], [1, 512]])
        nc.gpsimd.dma_gather(embT_ap, enc_tab[:, :], enc_idx[:, 32 * c:32 * (c + 1)],
                             512, 512, E,
                             elem_step=E, transpose=True).then_inc(s_gat_e, 16)
    gat_e_total = 16 * n_gchunk_e

    # relu(embT): split ACT / DVE
    nc.scalar.wait_ge(s_gat_e, gat_e_total)
    nc.vector.wait_ge(s_gat_e, gat_e_total)
    embT_f = AP(embT_h, 0, [[emb_pstride, 128], [1, 4 * ne]])
    half = 2 * ne
    nc.scalar.activation(embT_f[:, 0:half], embT_f[:, 0:half], AF.Relu).then_inc(s_relu, 1)
    nc.vector.tensor_relu(embT_f[:, half:2 * half], embT_f[:, half:2 * half]).then_inc(s_relu, 1)
    c_relu.add(2)
    relu_e_done = c_relu.v

    # ============ phase P1: encoder input gates ============
    nc.tensor.wait_ge(s_dma, loads_done)
    nc.tensor.wait_ge(s_relu, relu_e_done)

    def ig_precompute(nchunks, wT, nk, src_h, src_stride, src_pstride, ig_h, total_cols, with_bias, bias):
        evac_base = c_evac.v
        g = 0
        for n in range(nchunks):
            for m in range(4):
                bank = pb[g % 4]
                if g >= 4:
                    nc.tensor.wait_ge(s_evac, evac_base + g - 3)
                ins = None
                for k in range(nk):
                    ins = nc.tensor.matmul(
                        bank[:, 0:512],
                        wT[:, k * 512 + m * 128: k * 512 + (m + 1) * 128],
                        AP(src_h, (n * nk + k) * 512, [[src_pstride, 128], [1, 512]]),
                        start=(k == 0), stop=(k == nk - 1),
                    )
                ins.then_inc(s_pe, 1); c_pe.add(1)
                nc.scalar.wait_ge(s_pe, c_pe.v)
                out_ap = AP(ig_h, n * 2048 + m * 32, [[total_cols, 128], [128, 16], [1, 32]])
                if with_bias:
                    nc.scalar.activation(out_ap, bank[:, 0:512], AF.Identity,
                                         bias=bias[:, m:m + 1]).then_inc(s_evac, 1)
                else:
                    nc.scalar.activation(out_ap, bank[:, 0:512], AF.Copy).then_inc(s_evac, 1)
                c_evac.add(1)
                g += 1

    ig_precompute(ne // 512, enc_wT, 4, embT_h, ne, emb_pstride, ig_e_h, n_s * 128, True, enc_b)

    nc.all_engine_barrier()

    # dec gather now (embT region dead; PE finished reading at the barrier)
    n_gchunk_d = ntk // 512
    for c in range(n_gchunk_d):
        dembT_ap = AP(dembT_h, c * 8 * 512, [[8 * ntk, 128], [512, 8], [1, 512]])
        nc.gpsimd.dma_gather(dembT_ap, dec_tab[:, :], dec_idx[:, 32 * c:32 * (c + 1)],
                             512, 512, HD,
                             elem_step=HD, transpose=True).then_inc(s_gat_d, 16)
    gat_d_total = 16 * n_gchunk_d

    # ============ phase P2: encoder recurrence ============
    n_dchunk = 8
    relu_d_chunks = [AP(dembT_h, i * ntk, [[8 * ntk, 128], [1, ntk]])
                     for i in range(n_dchunk)]
    dembT_relu_done = Ctr()

    def emit_recurrence(steps, nk, npeer, hT, c_st, uT, ig,
                        write_out, relu_sched):
        pe_base = c_pe.v
        add_base = c_add.v
        sig_base = c_sig.v
        cup_base = c_cup.v
        tc_base = c_tc.v
        cast_base = c_cast.v
        for t in range(steps):
            par, nxt = t % 2, (t + 1) % 2
            bank = pb[par]
            # --- PE ---
            nc.tensor.wait_ge(s_cast, cast_base + t)
            for j in range(1, npeer + 1):
                if recv_cnt[(j, par)] > 0:
                    nc.tensor.wait_ge(s_recv[(j, par)], recv_cnt[(j, par)])
            if t >= 2:
                nc.tensor.wait_ge(s_add, add_base + t - 1)
            for m in range(4):
                ins = None
                for k in range(nk):
                    ins = nc.tensor.matmul(
                        bank[:, m * 32:(m + 1) * 32],
                        uT[:, k * 512 + m * 128: k * 512 + (m + 1) * 128],
                        hT[par][:, k * 32:(k + 1) * 32],
                        start=(k == 0), stop=(k == nk - 1),
                    )
                if m == 3:
                    ins.then_inc(s_pe, 1); c_pe.add(1)
            # --- DVE: pre = psum + ig[t] ---
            nc.vector.wait_ge(s_pe, pe_base + t + 1)
            nc.vector.tensor_add(pre[:, :], bank[:, 0:128],
                                 ig[:, t * 128:(t + 1) * 128]).then_inc(s_add, 1)
            c_add.add(1)
            # --- ACT: sigmoid(i,f,o), tanh(g) ---
            nc.scalar.wait_ge(s_add, add_base + t + 1)
            nc.scalar.activation(act[:, 0:96], pre[:, 0:96], AF.Sigmoid)
            nc.scalar.activation(act[:, 96:128], pre[:, 96:128], AF.Tanh).then_inc(s_sig, 1)
            c_sig.add(1)
            # --- DVE: c = f*c + i*g ---
            nc.vector.wait_ge(s_sig, sig_base + t + 1)
            if t == 0:
                nc.vector.wait_ge(s_misc, memsets_done)
            else:
                nc.vector.wait_ge(s_cup, cup_base + t)  # c[par] write drained
            nc.vector.tensor_mul(m1[:, :], act[:, 0:32], act[:, 96:128]).then_inc(s_dvef, 1)
            c_dvef.add(1)
            nc.vector.tensor_mul(m2[:, :], act[:, 32:64], c_st[par][:, :]).then_inc(s_dvef, 1)
            c_dvef.add(1)
            nc.vector.wait_ge(s_dvef, c_dvef.v)
            nc.vector.tensor_add(c_st[nxt][:, :], m1[:, :], m2[:, :]).then_inc(s_cup, 1)
            c_cup.add(1)
            # --- ACT: tanh(c) ---
            nc.scalar.wait_ge(s_cup, cup_base + t + 1)
            nc.scalar.activation(tc[:, :], c_st[nxt][:, :], AF.Tanh).then_inc(s_tc, 1)
            c_tc.add(1)
            # --- DVE: h = o * tanh(c) (+ bf16 cast into own send slot) ---
            nc.vector.wait_ge(s_tc, tc_base + t + 1)
            if loc_cnt[nxt] > 0:
                nc.vector.wait_ge(s_loc[nxt], loc_cnt[nxt])
            if write_out is not None:
                nc.vector.tensor_mul(write_out[:, t * 32:(t + 1) * 32],
                                     act[:, 64:96], tc[:, :]).then_inc(s_dvef, 1)
                c_dvef.add(1)
                nc.vector.wait_ge(s_dvef, c_dvef.v)
                nc.vector.tensor_copy(hT[nxt][:, 0:32],
                                      write_out[:, t * 32:(t + 1) * 32]).then_inc(s_cast, 1)
            else:
                nc.vector.tensor_mul(hT[nxt][:, 0:32], act[:, 64:96],
                                     tc[:, :]).then_inc(s_cast, 1)
            c_cast.add(1)
            if relu_sched is not None and t in relu_sched:
                ci = relu_sched[t]
                nc.vector.wait_ge(s_gat_d, gat_d_total)
                nc.vector.tensor_relu(relu_d_chunks[ci], relu_d_chunks[ci]).then_inc(s_relu, 1)
                c_relu.add(1)
                dembT_relu_done.v = c_relu.v
            # --- GPS: broadcast h slice ---
            if t < steps - 1:
                for j in range(1, npeer + 1):
                    rdests = [None] * 8
                    rdests[j] = (0, j)
                    nc.gpsimd.remote_dma_broadcast(
                        out_ap=hT[nxt][:, j * 32:(j + 1) * 32],
                        in_ap=hT[nxt][:, 0:32],
                        remote_sem=s_recv[(j, nxt)], local_sem=s_loc[nxt],
                        rdests=rdests,
                    ).then_inc(s_prep, 1)
                    c_prep.add(1)
                    recv_cnt[(j, nxt)] += 2
                loc_cnt[nxt] += 16 * npeer
                nc.gpsimd.wait_ge(s_prep, c_prep.v)
                nc.gpsimd.wait_ge(s_cast, cast_base + t + 1)
                nc.gpsimd.trigger_dma(count=npeer)

    if n_s >= 64:
        relu_sched = {32 + 2 * i: i for i in range(n_dchunk)}
    else:
        relu_sched = {i: i for i in range(n_dchunk)}
        assert n_s >= n_dchunk
    emit_recurrence(n_s, 4, 3, hT_e, c_e, enc_uT, ig_e, None, relu_sched)

    # ---- encoder final -> decoder h0 exchange ----
    final_par = n_s % 2
    nc.vector.wait_ge(s_cast, c_cast.v)
    nc.vector.tensor_copy(hT_d[0][:, 0:32], hT_e[final_par][:, 0:32]).then_inc(s_cast, 1)
    c_cast.add(1)
    cast_init_d = c_cast.v
    for j in range(1, 8):
        rdests = [None] * 8
        rdests[j] = (0, j)
        nc.gpsimd.remote_dma_broadcast(
            out_ap=hT_d[0][:, j * 32:(j + 1) * 32], in_ap=hT_d[0][:, 0:32],
            remote_sem=s_recv[(j, 0)], local_sem=s_loc[0], rdests=rdests,
        ).then_inc(s_prep, 1)
        c_prep.add(1)
        recv_cnt[(j, 0)] += 2
    loc_cnt[0] += 112
    nc.gpsimd.wait_ge(s_prep, c_prep.v)
    nc.gpsimd.wait_ge(s_cast, cast_init_d)
    nc.gpsimd.trigger_dma(count=7)

    nc.all_engine_barrier()

    # ============ phase P3: decoder input gates (demb part) ============
    assert dembT_relu_done.v > 0
    nc.tensor.wait_ge(s_relu, dembT_relu_done.v)
    ig_precompute(ntk // 512, dec_w1T, 8, dembT_h, ntk, 8 * ntk, ig_d_h, n_t * 128, False, None)

    # ============ phase P4: ctx gates + fold into ig_d ============
    for j in range(1, 8):
        nc.tensor.wait_ge(s_recv[(j, 0)], recv_cnt[(j, 0)])
    nc.tensor.wait_ge(s_cast, cast_init_d)
    evac_base4 = c_evac.v
    for m in range(4):
        bank = pb[m]
        nc.tensor.wait_ge(s_evac, c_evac.v)  # banks were used by P3 tail
        ins = None
        for k in range(8):
            ins = nc.tensor.matmul(
                bank[:, 0:32],
                dec_w2T[:, k * 512 + m * 128: k * 512 + (m + 1) * 128],
                hT_d[0][:, k * 32:(k + 1) * 32],
                start=(k == 0), stop=(k == 7),
            )
        ins.then_inc(s_pe, 1); c_pe.add(1)
        nc.scalar.wait_ge(s_pe, c_pe.v)
        nc.scalar.activation(ctx_g[:, m * 32:(m + 1) * 32], bank[:, 0:32],
                             AF.Identity, bias=dec_b[:, m:m + 1]).then_inc(s_evac, 1)
        c_evac.add(1)
    nc.vector.wait_ge(s_evac, c_evac.v)
    ctx_rep = AP(ctx_g_h, 0, [[128, 128], [0, n_t], [1, 128]])
    igd_3d = AP(ig_d_h, 0, [[n_t * 128, 128], [128, n_t], [1, 128]])
    nc.vector.tensor_add(igd_3d, igd_3d, ctx_rep).then_inc(s_add, 1)
    c_add.add(1)

    nc.all_engine_barrier()

    # ============ phase P5: decoder recurrence ============
    emit_recurrence(n_t, 8, 7, hT_d, c_d, dec_uT, ig_d, outT, None)

    # ============ output ============
    nc.sync.wait_ge(s_cast, c_cast.v)
    nc.sync.dma_start(out=out_d[:], in_=outT[:, :]).then_inc(s_dma, 16)
    c_dma.add(16)
    nc.sync.wait_ge(s_dma, c_dma.v)

    nc.compile()
    return nc


# ---------------------------------------------------------------------------
# host-side data prep
# ---------------------------------------------------------------------------

def _wrap_idx(idx_flat):
    n = idx_flat.shape[0]
    w = idx_flat.astype(np.int16).reshape(n // 16, 16).T
    return np.ascontiguousarray(np.tile(w, (8, 1)))


def _wT_sbuf(WT, chunk_rows, gate_cols):
    """WT: [Din, 4H] (= W.T); -> [128, nk*512] bf16 SBUF layout."""
    nk = len(chunk_rows)
    out = np.empty((128, nk * 512), NP_BF16)
    for j, r0 in enumerate(chunk_rows):
        out[:, j * 512:(j + 1) * 512] = WT[r0:r0 + 128][:, gate_cols].astype(NP_BF16)
    return out


def _gate_cols(hs, H):
    cols = np.empty(512, np.int64)
    for m in range(4):
        g = _GATE_BLOCK[m]
        cols[m * 128:(m + 1) * 128] = np.arange(hs, hs + 128) + g * H
    return cols


def prepare_in_maps(inputs, slot_map, n_s=S, n_t=T):
    tokens = np.asarray(inputs["tokens"]).astype(np.int64)[:, :n_s]
    trg = np.asarray(inputs["trg_seqs"]).astype(np.int64)[:, :n_t]
    dec_in = np.concatenate([np.full((B, 1), 1, np.int64), trg[:, :-1]], axis=1)

    enc_tab = np.asarray(inputs["enc_emb"]).astype(NP_BF16)
    dec_tab = np.asarray(inputs["dec_emb"]).astype(NP_BF16)

    quadA = sorted(int(x) for x in slot_map[0, :4])
    quadB = sorted(int(x) for x in set(range(8)) - set(quadA))
    is_fwd = {c: (c in quadA) for c in range(8)}
    qrank = {}
    for q in (quadA, quadB):
        for a, c in enumerate(q):
            qrank[c] = a
    enc_rows = {c: (qrank[c] * 128 if is_fwd[c] else 512 + qrank[c] * 128)
                for c in range(8)}

    WihT = {True: np.asarray(inputs["enc_Wih_f"]).T, False: np.asarray(inputs["enc_Wih_b"]).T}
    WhhT = {True: np.asarray(inputs["enc_Whh_f"]).T, False: np.asarray(inputs["enc_Whh_b"]).T}
    enc_bias = {True: np.asarray(inputs["enc_b_f"]), False: np.asarray(inputs["enc_b_b"])}
    W1T = np.asarray(inputs["dec_Wih"])[:, :HD].T
    W2T = np.asarray(inputs["dec_Wih"])[:, HD:].T
    UT = np.asarray(inputs["dec_Whh"]).T
    db = np.asarray(inputs["dec_b"])

    dec_idx_w = _wrap_idx(dec_in.T.reshape(-1))

    in_maps = []
    for r in range(8):
        fwd = is_fwd[r]
        hs = qrank[r] * 128
        gcols_e = _gate_cols(hs, E)
        enc_wT = _wT_sbuf(WihT[fwd], [0, 128, 256, 384], gcols_e)
        chunk_rows = [qrank[int(slot_map[r, j])] * 128 for j in range(4)]
        enc_uT = _wT_sbuf(WhhT[fwd], chunk_rows, gcols_e)
        eb = np.empty((128, 4), np.float32)
        for m in range(4):
            g = _GATE_BLOCK[m]
            eb[:, m] = enc_bias[fwd][g * E + hs: g * E + hs + 128]
        tk = tokens if fwd else tokens[:, ::-1]
        enc_idx_w = _wrap_idx(tk.T.reshape(-1))

        hs_d = r * 128
        gcols_d = _gate_cols(hs_d, HD)
        dec_w1T = _wT_sbuf(W1T, [128 * k for k in range(8)], gcols_d)
        w2_rows = [enc_rows[int(slot_map[r, j])] for j in range(8)]
        dec_w2T = _wT_sbuf(W2T, w2_rows, gcols_d)
        u_rows = [int(slot_map[r, j]) * 128 for j in range(8)]
        dec_uT = _wT_sbuf(UT, u_rows, gcols_d)
        dbv = np.empty((128, 4), np.float32)
        for m in range(4):
            g = _GATE_BLOCK[m]
            dbv[:, m] = db[g * HD + hs_d: g * HD + hs_d + 128]

        in_maps.append(dict(
            enc_tab=enc_tab, dec_tab=dec_tab,
            enc_idx=enc_idx_w, dec_idx=dec_idx_w,
            enc_wT=enc_wT, enc_uT=enc_uT, enc_b=eb,
            dec_w1T=dec_w1T, dec_w2T=dec_w2T, dec_uT=dec_uT, dec_b=dbv,
        ))
    return in_maps


def assemble_output(results, n_t=T):
    decoded = np.empty((B, n_t, HD), np.float32)
    for r in range(8):
        o = np.asarray(results[r]["out"])  # [128, n_t*32]
        decoded[:, :, r * 128:(r + 1) * 128] = (
            o.reshape(128, n_t, 32).transpose(2, 1, 0)
        )
    return decoded


_KERNEL_CACHE = {}


def _get_kernel(n_s, n_t):
    key = (n_s, n_t)
    if key not in _KERNEL_CACHE:
        _KERNEL_CACHE[key] = build_kernel(n_s, n_t)
    return _KERNEL_CACHE[key]


def run(inputs, n_s=S, n_t=T, trace=False):
    slot_map = get_slot_map()
    nc = _get_kernel(n_s, n_t)
    in_maps = prepare_in_maps(inputs, slot_map, n_s, n_t)
    res = run_bass_kernel_spmd(nc, in_maps, core_ids=list(range(NC)), trace=trace)
    return assemble_output(res.results, n_t), res


def kernel(**inputs) -> np.ndarray:
    out, _ = run(inputs)
    return out



# revision 9
# speedup vs baseline: 1.0201x; 1.0201x over previous
"""Trainium2 Bass kernel for nn_Decoder (seq2seq BiLSTM encoder + LSTM decoder).

Strategy (8 NeuronCores, SPMD):
  - Gate/hidden-dim model parallelism for both recurrences:
      encoder: fwd chain on one physical quad (4 cores), bwd chain on the
      other; each core owns a 128-row hidden slice (all 4 gates).
      decoder: all 8 cores, each owns a 128-row slice of the 1024 hidden.
  - Per-step h exchange via remote_dma_broadcast (SBUF->SBUF, XOR-relative
    dests).  Receiver slot j holds the slice of the core at phys XOR j; the
    host permutes weight K-chunks per core to match (slot map discovered
    once by a probe kernel).
  - Input-to-hidden projections precomputed for all timesteps as batched
    matmuls (weight-stationary bf16 / FWL).
  - Embedding lookups via dma_gather(transpose=True) directly into the
    transposed [feature, token] layout.
  - Matmuls bf16 -> fp32 PSUM; c state fp32; h carried bf16
    (validated: global rel err ~3e-3 vs fp32 reference).
"""

import sys
import numpy as np
import ml_dtypes

for _p in ("/opt/trn_rl_repo",):
    if _p not in sys.path:
        sys.path.insert(0, _p)

import concourse.bass as bass
import concourse.bacc as bacc
import concourse.mybir as mybir
from concourse.bass import AP
from concourse.bass_utils import run_bass_kernel_spmd

BF16 = mybir.dt.bfloat16
F32 = mybir.dt.float32
NP_BF16 = ml_dtypes.bfloat16

E = 512
V = 32000
B = 32
S = 128
T = 128
HD = 2 * E
NC = 8
AF = mybir.ActivationFunctionType

# gate col order within each 128-block: [i, f, o, g]; pytorch rows: i,f,g,o
_GATE_BLOCK = {0: 0, 1: 1, 2: 3, 3: 2}


class Ctr:
    __slots__ = ("v",)
    def __init__(self):
        self.v = 0
    def add(self, n):
        self.v += n
        return self.v


def _build_probe():
    nc = bacc.Bacc(None, target_bir_lowering=False, num_devices=NC)
    myid = nc.dram_tensor("myid", [128, 32], F32, kind="ExternalInput")
    out = nc.dram_tensor("out", [128, 8 * 32], F32, kind="ExternalOutput")
    recv_sem = nc.alloc_semaphore("recv_sem")
    local_sem = nc.alloc_semaphore("local_sem")
    prep_sem = nc.alloc_semaphore("prep_sem")
    dma_sem = nc.alloc_semaphore("dma_sem")
    own = nc.alloc_sbuf_tensor("own", [128, 32], F32).ap()
    recv = nc.alloc_sbuf_tensor("recv", [128, 8 * 32], F32).ap()
    nc.gpsimd.memset(recv[:, :], -1.0)
    nc.sync.dma_start(out=own[:, :], in_=myid[:]).then_inc(dma_sem, 16)
    nc.all_core_barrier()
    nc.gpsimd.wait_ge(dma_sem, 16)
    nc.vector.tensor_copy(recv[:, 0:32], own[:, :]).then_inc(prep_sem, 1)
    for j in range(1, 8):
        rdests = [None] * 8
        rdests[j] = (0, j)
        nc.gpsimd.remote_dma_broadcast(
            out_ap=recv[:, j * 32:(j + 1) * 32], in_ap=own[:, :],
            remote_sem=recv_sem, local_sem=local_sem, rdests=rdests,
        ).then_inc(prep_sem, 1)
    nc.gpsimd.wait_ge(prep_sem, 8)
    nc.gpsimd.trigger_dma(count=7)
    nc.sync.wait_ge(recv_sem, 14)
    nc.sync.dma_start(out=out[:], in_=recv[:]).then_inc(dma_sem, 16)
    nc.sync.wait_ge(dma_sem, 32)
    nc.compile()
    return nc


_SLOT_MAP = None


def get_slot_map():
    """slot_map[r][j] = logical core whose broadcast lands in slot j on core r."""
    global _SLOT_MAP
    if _SLOT_MAP is not None:
        return _SLOT_MAP
    nc = _build_probe()
    in_maps = [{"myid": np.full((128, 32), float(c), np.float32)} for c in range(NC)]
    res = run_bass_kernel_spmd(nc, in_maps, core_ids=list(range(NC)))
    sm = np.zeros((NC, 8), np.int64)
    for r in range(NC):
        o = res.results[r]["out"]
        for j in range(8):
            v = o[:, j * 32:(j + 1) * 32]
            assert (v == v[0, 0]).all(), f"probe: core {r} slot {j} non-uniform"
            sm[r, j] = int(v[0, 0])
    assert (sm[:, 0] == np.arange(NC)).all(), sm
    for r in range(NC):
        assert sorted(sm[r]) == list(range(8)), sm[r]
        for j in range(8):
            assert sm[sm[r, j], j] == r, (r, j)
    _SLOT_MAP = sm
    return sm


# ---------------------------------------------------------------------------
# kernel builder
# ---------------------------------------------------------------------------

def build_kernel(n_s=S, n_t=T):
    assert (B * n_s) % 512 == 0 and (B * n_t) % 512 == 0
    nc = bacc.Bacc(None, target_bir_lowering=False, num_devices=NC,
                   dynamic_dma_scratch_size=32768, num_swdge_queues=4)
    ne, ntk = B * n_s, B * n_t

    # ---------------- I/O ----------------
    enc_tab = nc.dram_tensor("enc_tab", [V, E], BF16, kind="ExternalInput")
    dec_tab = nc.dram_tensor("dec_tab", [V, HD], BF16, kind="ExternalInput")
    enc_idx_d = nc.dram_tensor("enc_idx", [128, ne // 16], mybir.dt.int16, kind="ExternalInput")
    dec_idx_d = nc.dram_tensor("dec_idx", [128, ntk // 16], mybir.dt.int16, kind="ExternalInput")
    enc_wT_d = nc.dram_tensor("enc_wT", [128, 4 * 512], BF16, kind="ExternalInput")
    enc_uT_d = nc.dram_tensor("enc_uT", [128, 4 * 512], BF16, kind="ExternalInput")
    enc_b_d = nc.dram_tensor("enc_b", [128, 4], F32, kind="ExternalInput")
    dec_w1T_d = nc.dram_tensor("dec_w1T", [128, 8 * 512], BF16, kind="ExternalInput")
    dec_w2T_d = nc.dram_tensor("dec_w2T", [128, 8 * 512], BF16, kind="ExternalInput")
    dec_uT_d = nc.dram_tensor("dec_uT", [128, 8 * 512], BF16, kind="ExternalInput")
    dec_b_d = nc.dram_tensor("dec_b", [128, 4], F32, kind="ExternalInput")
    out_d = nc.dram_tensor("out", [128, n_t * 32], F32, kind="ExternalOutput")

    # ---------------- SBUF ----------------
    sb = nc.alloc_sbuf_tensor
    enc_wT = sb("enc_wT_s", [128, 4 * 512], BF16).ap()
    enc_uT = sb("enc_uT_s", [128, 4 * 512], BF16).ap()
    enc_b = sb("enc_b_s", [128, 4], F32).ap()
    dec_w1T = sb("dec_w1T_s", [128, 8 * 512], BF16).ap()
    dec_w2T = sb("dec_w2T_s", [128, 8 * 512], BF16).ap()
    dec_uT = sb("dec_uT_s", [128, 8 * 512], BF16).ap()
    dec_b = sb("dec_b_s", [128, 4], F32).ap()
    enc_idx = sb("enc_idx_s", [128, ne // 16], mybir.dt.int16).ap()
    dec_idx = sb("dec_idx_s", [128, ntk // 16], mybir.dt.int16).ap()
    dembT_h = sb("dembT", [128, 8 * ntk], BF16)    # [128][8][ntk]
    # encoder embT aliases the first 4*ne cols of dembT (dead before dec gather)
    if 4 * ne <= 8 * ntk:
        embT_h = dembT_h
        emb_pstride = 8 * ntk
    else:
        embT_h = sb("embT", [128, 4 * ne], BF16)
        emb_pstride = 4 * ne
    ig_e_h = sb("ig_e", [128, n_s * 128], BF16)    # col t*128 + m*32 + b
    ig_d_h = sb("ig_d", [128, n_t * 128], BF16)
    ig_e = ig_e_h.ap()
    ig_d = ig_d_h.ap()
    hT_e = [sb(f"hT_e{p}", [128, 4 * 32], BF16).ap() for p in range(2)]
    hT_d = [sb(f"hT_d{p}", [128, 8 * 32], BF16).ap() for p in range(2)]
    c_e = [sb(f"c_e{p}", [128, 32], F32).ap() for p in range(2)]
    c_d = [sb(f"c_d{p}", [128, 32], F32).ap() for p in range(2)]
    pre = sb("pre", [128, 128], F32).ap()
    act = sb("act", [128, 128], F32).ap()
    tc = sb("tc", [128, 32], F32).ap()
    m1 = sb("m1", [128, 32], F32).ap()
    m2 = sb("m2", [128, 32], F32).ap()
    ctx_g_h = sb("ctx_g", [128, 128], F32)
    ctx_g = ctx_g_h.ap()
    outT = sb("outT", [128, n_t * 32], F32).ap()

    pb = [nc.alloc_psum_tensor(f"pb{i}", [128, 512], F32).ap() for i in range(4)]

    # ---------------- semaphores ----------------
    sem = nc.alloc_semaphore
    s_dma = sem("s_dma");   c_dma = Ctr()
    s_gat_e = sem("s_gat_e")
    s_gat_d = sem("s_gat_d")
    s_relu = sem("s_relu"); c_relu = Ctr()
    s_pe = sem("s_pe");     c_pe = Ctr()
    s_evac = sem("s_evac"); c_evac = Ctr()
    s_add = sem("s_add");   c_add = Ctr()
    s_sig = sem("s_sig");   c_sig = Ctr()
    s_cup = sem("s_cup");   c_cup = Ctr()
    s_tc = sem("s_tc");     c_tc = Ctr()
    s_cast = sem("s_cast"); c_cast = Ctr()
    s_prep = sem("s_prep"); c_prep = Ctr()
    # per-slot, per-buffer-parity receive sems (shared by enc/dec phases);
    # thresholds are taken from emission-time counters, which makes the
    # cumulative waits race-free (see design notes).
    s_recv = {(j, p): sem(f"s_recv_{j}_{p}") for j in range(1, 8) for p in range(2)}
    recv_cnt = {k: 0 for k in s_recv}
    s_loc = [sem("s_loc_0"), sem("s_loc_1")]
    loc_cnt = [0, 0]
    s_misc = sem("s_misc"); c_misc = Ctr()
    s_dvef = sem("s_dvef"); c_dvef = Ctr()
    s_pe3 = sem("s_pe3"); c_pe3 = Ctr()

    def dma(dst, src):
        nc.sync.dma_start(out=dst, in_=src).then_inc(s_dma, 16)
        c_dma.add(16)

    # ============ phase L: loads + init ============
    dma(enc_wT[:, :], enc_wT_d[:])
    dma(enc_uT[:, :], enc_uT_d[:])
    dma(enc_b[:, :], enc_b_d[:])
    dma(dec_w1T[:, :], dec_w1T_d[:])
    dma(dec_w2T[:, :], dec_w2T_d[:])
    dma(dec_uT[:, :], dec_uT_d[:])
    dma(dec_b[:, :], dec_b_d[:])
    dma(enc_idx[:, :], enc_idx_d[:])
    dma(dec_idx[:, :], dec_idx_d[:])
    loads_done = c_dma.v

    nc.gpsimd.memset(hT_e[0][:, :], 0.0).then_inc(s_misc, 1); c_misc.add(1)
    nc.gpsimd.memset(c_e[0][:, :], 0.0).then_inc(s_misc, 1); c_misc.add(1)
    nc.gpsimd.memset(c_d[0][:, :], 0.0).then_inc(s_misc, 1); c_misc.add(1)
    memsets_done = c_misc.v

    nc.all_core_barrier()

    # ============ phase G: gathers ============
    nc.gpsimd.wait_ge(s_dma, loads_done)
    n_gchunk_e = ne // 512
    for c in range(n_gchunk_e):
        embT_ap = AP(embT_h, c * 4 * 512, [[emb_pstride, 128], [512, 4], [1, 512]])
        nc.gpsimd.dma_gather(embT_ap, enc_tab[:, :], enc_idx[:, 32 * c:32 * (c + 1)],
                             512, 512, E,
                             elem_step=E, transpose=True).then_inc(s_gat_e, 16)
    gat_e_total = 16 * n_gchunk_e

    # relu(embT): split ACT / DVE
    nc.scalar.wait_ge(s_gat_e, gat_e_total)
    nc.vector.wait_ge(s_gat_e, gat_e_total)
    embT_f = AP(embT_h, 0, [[emb_pstride, 128], [1, 4 * ne]])
    half = 2 * ne
    nc.scalar.activation(embT_f[:, 0:half], embT_f[:, 0:half], AF.Relu).then_inc(s_relu, 1)
    nc.vector.tensor_relu(embT_f[:, half:2 * half], embT_f[:, half:2 * half]).then_inc(s_relu, 1)
    c_relu.add(2)
    relu_e_done = c_relu.v

    # ============ phase P1: encoder input gates ============
    nc.tensor.wait_ge(s_dma, loads_done)
    nc.tensor.wait_ge(s_relu, relu_e_done)

    def ig_precompute(nchunks, wT, nk, src_h, src_stride, src_pstride, ig_h, total_cols, with_bias, bias):
        evac_base = c_evac.v
        g = 0
        for n in range(nchunks):
            for m in range(4):
                bank = pb[g % 4]
                if g >= 4:
                    nc.tensor.wait_ge(s_evac, evac_base + g - 3)
                ins = None
                for k in range(nk):
                    ins = nc.tensor.matmul(
                        bank[:, 0:512],
                        wT[:, k * 512 + m * 128: k * 512 + (m + 1) * 128],
                        AP(src_h, (n * nk + k) * 512, [[src_pstride, 128], [1, 512]]),
                        start=(k == 0), stop=(k == nk - 1),
                    )
                ins.then_inc(s_pe, 1); c_pe.add(1)
                nc.scalar.wait_ge(s_pe, c_pe.v)
                out_ap = AP(ig_h, n * 2048 + m * 32, [[total_cols, 128], [128, 16], [1, 32]])
                if with_bias:
                    nc.scalar.activation(out_ap, bank[:, 0:512], AF.Identity,
                                         bias=bias[:, m:m + 1]).then_inc(s_evac, 1)
                else:
                    nc.scalar.activation(out_ap, bank[:, 0:512], AF.Copy).then_inc(s_evac, 1)
                c_evac.add(1)
                g += 1

    ig_precompute(ne // 512, enc_wT, 4, embT_h, ne, emb_pstride, ig_e_h, n_s * 128, True, enc_b)

    nc.all_engine_barrier()

    # dec gather now (embT region dead; PE finished reading at the barrier)
    n_gchunk_d = ntk // 512
    for c in range(n_gchunk_d):
        dembT_ap = AP(dembT_h, c * 8 * 512, [[8 * ntk, 128], [512, 8], [1, 512]])
        nc.gpsimd.dma_gather(dembT_ap, dec_tab[:, :], dec_idx[:, 32 * c:32 * (c + 1)],
                             512, 512, HD,
                             elem_step=HD, transpose=True).then_inc(s_gat_d, 16)
    gat_d_total = 16 * n_gchunk_d

    # ============ phase P2: encoder recurrence ============
    n_dchunk = 8
    relu_d_chunks = [AP(dembT_h, i * ntk, [[8 * ntk, 128], [1, ntk]])
                     for i in range(n_dchunk)]
    dembT_relu_done = Ctr()
    relu_marks = {}          # token-block -> s_relu threshold once relu'd

    # --- P3 (decoder input-gate precompute) interleaved into the encoder
    # recurrence: one (n, m) group of 8 [128,128,512] matmuls per slot, on
    # psum banks 2/3 and a dedicated s_pe3 sem so the recurrence's s_pe
    # arithmetic is untouched.  Evac is deferred one step so the ACT queue
    # never stalls on a pending s_pe3 wait.
    p3_groups = [(n, m) for n in range(ntk // 512) for m in range(4)]
    p3_evac_pending = []

    def p3_mms(g):
        n, m = p3_groups[g]
        bank = pb[2 + g % 2]
        nc.tensor.wait_ge(s_relu, relu_marks[n])
        if g >= 2:
            nc.tensor.wait_ge(s_evac, p3_evac_base + g - 1)
        ins = None
        for k in range(8):
            ins = nc.tensor.matmul(
                bank[:, 0:512],
                dec_w1T[:, k * 512 + m * 128: k * 512 + (m + 1) * 128],
                AP(dembT_h, (n * 8 + k) * 512, [[8 * ntk, 128], [1, 512]]),
                start=(k == 0), stop=(k == 7),
            )
        ins.then_inc(s_pe3, 1); c_pe3.add(1)
        p3_evac_pending.append((g, c_pe3.v))

    def p3_evac_flush():
        while p3_evac_pending:
            g, pe3_thr = p3_evac_pending.pop(0)
            n, m = p3_groups[g]
            bank = pb[2 + g % 2]
            nc.scalar.wait_ge(s_pe3, pe3_thr)
            out_ap = AP(ig_d_h, n * 2048 + m * 32,
                        [[n_t * 128, 128], [128, 16], [1, 32]])
            nc.scalar.activation(out_ap, bank[:, 0:512], AF.Copy).then_inc(s_evac, 1)
            c_evac.add(1)

    def emit_recurrence(steps, nk, npeer, hT, c_st, uT, ig,
                        write_out, relu_sched, bcast_queue, p3_sched=None):
        pe_base = c_pe.v
        add_base = c_add.v
        sig_base = c_sig.v
        cup_base = c_cup.v
        tc_base = c_tc.v
        cast_base = c_cast.v
        for t in range(steps):
            par, nxt = t % 2, (t + 1) % 2
            bank = pb[par]
            # --- PE ---
            nc.tensor.wait_ge(s_cast, cast_base + t)
            for j in range(1, npeer + 1):
                if recv_cnt[(j, par)] > 0:
                    nc.tensor.wait_ge(s_recv[(j, par)], recv_cnt[(j, par)])
            if t >= 2:
                nc.tensor.wait_ge(s_add, add_base + t - 1)
            for m in range(4):
                ins = None
                for k in range(nk):
                    ins = nc.tensor.matmul(
                        bank[:, m * 32:(m + 1) * 32],
                        uT[:, k * 512 + m * 128: k * 512 + (m + 1) * 128],
                        hT[par][:, k * 32:(k + 1) * 32],
                        start=(k == 0), stop=(k == nk - 1),
                    )
                if m == 3:
                    ins.then_inc(s_pe, 1); c_pe.add(1)
            if p3_sched is not None and t in p3_sched:
                p3_mms(p3_sched[t])
            # --- DVE: pre = psum + ig[t] ---
            nc.vector.wait_ge(s_pe, pe_base + t + 1)
            nc.vector.tensor_add(pre[:, :], bank[:, 0:128],
                                 ig[:, t * 128:(t + 1) * 128]).then_inc(s_add, 1)
            c_add.add(1)
            # --- ACT: sigmoid(i,f,o), tanh(g) ---
            nc.scalar.wait_ge(s_add, add_base + t + 1)
            nc.scalar.activation(act[:, 0:96], pre[:, 0:96], AF.Sigmoid)
            nc.scalar.activation(act[:, 96:128], pre[:, 96:128], AF.Tanh).then_inc(s_sig, 1)
            c_sig.add(1)
            if p3_sched is not None and (t - 1) in p3_sched:
                p3_evac_flush()
            # --- DVE: c = f*c + i*g ---
            nc.vector.wait_ge(s_sig, sig_base + t + 1)
            if t == 0:
                nc.vector.wait_ge(s_misc, memsets_done)
            else:
                nc.vector.wait_ge(s_cup, cup_base + t)  # c[par] write drained
            nc.vector.tensor_mul(m1[:, :], act[:, 0:32], act[:, 96:128]).then_inc(s_dvef, 1)
            c_dvef.add(1)
            nc.vector.tensor_mul(m2[:, :], act[:, 32:64], c_st[par][:, :]).then_inc(s_dvef, 1)
            c_dvef.add(1)
            nc.vector.wait_ge(s_dvef, c_dvef.v)
            nc.vector.tensor_add(c_st[nxt][:, :], m1[:, :], m2[:, :]).then_inc(s_cup, 1)
            c_cup.add(1)
            # --- ACT: tanh(c) ---
            nc.scalar.wait_ge(s_cup, cup_base + t + 1)
            nc.scalar.activation(tc[:, :], c_st[nxt][:, :], AF.Tanh).then_inc(s_tc, 1)
            c_tc.add(1)
            # --- DVE: h = o * tanh(c) (+ bf16 cast into own send slot) ---
            nc.vector.wait_ge(s_tc, tc_base + t + 1)
            if loc_cnt[nxt] > 0:
                nc.vector.wait_ge(s_loc[nxt], loc_cnt[nxt])
            if write_out is not None:
                nc.vector.tensor_mul(write_out[:, t * 32:(t + 1) * 32],
                                     act[:, 64:96], tc[:, :]).then_inc(s_dvef, 1)
                c_dvef.add(1)
                nc.vector.wait_ge(s_dvef, c_dvef.v)
                nc.vector.tensor_copy(hT[nxt][:, 0:32],
                                      write_out[:, t * 32:(t + 1) * 32]).then_inc(s_cast, 1)
            else:
                nc.vector.tensor_mul(hT[nxt][:, 0:32], act[:, 64:96],
                                     tc[:, :]).then_inc(s_cast, 1)
            c_cast.add(1)
            if relu_sched is not None and t in relu_sched:
                ci = relu_sched[t]
                nc.vector.wait_ge(s_gat_d, gat_d_total)
                nc.vector.tensor_relu(relu_d_chunks[ci], relu_d_chunks[ci]).then_inc(s_relu, 1)
                c_relu.add(1)
                relu_marks[ci] = c_relu.v
                dembT_relu_done.v = c_relu.v
            # --- GPS: broadcast h slice (spread over SWDGE queues) ---
            if t < steps - 1:
                qcnt = {}
                for j in range(1, npeer + 1):
                    rdests = [None] * 8
                    rdests[j] = (0, j)
                    q = bcast_queue[j]
                    nc.gpsimd.remote_dma_broadcast(
                        out_ap=hT[nxt][:, j * 32:(j + 1) * 32],
                        in_ap=hT[nxt][:, 0:32],
                        remote_sem=s_recv[(j, nxt)], local_sem=s_loc[nxt],
                        rdests=rdests, queue_num=q,
                    ).then_inc(s_prep, 1)
                    c_prep.add(1)
                    recv_cnt[(j, nxt)] += 2
                    qcnt[q] = qcnt.get(q, 0) + 1
                loc_cnt[nxt] += 16 * npeer
                nc.gpsimd.wait_ge(s_prep, c_prep.v)
                nc.gpsimd.wait_ge(s_cast, cast_base + t + 1)
                for q, cq in sorted(qcnt.items()):
                    nc.gpsimd.trigger_dma(count=cq, queue_num=q)

    if n_s >= 64:
        relu_sched = {32 + 2 * i: i for i in range(n_dchunk)}
        assert n_s >= 48 + 2 * len(p3_groups) + 2
        p3_sched = {48 + 2 * g: g for g in range(len(p3_groups))}
    else:
        relu_sched = {i: i for i in range(n_dchunk)}
        assert n_s >= n_dchunk
        p3_sched = None
    p3_evac_base = c_evac.v
    ENC_Q = {1: 1, 2: 2, 3: 3}
    DEC_Q = {1: 1, 2: 2, 3: 3, 4: 0, 5: 1, 6: 2, 7: 3}
    emit_recurrence(n_s, 4, 3, hT_e, c_e, enc_uT, ig_e, None, relu_sched,
                    ENC_Q, p3_sched)
    if p3_sched is not None:
        p3_evac_flush()

    # ---- encoder final -> decoder h0 exchange ----
    final_par = n_s % 2
    nc.vector.wait_ge(s_cast, c_cast.v)
    nc.vector.tensor_copy(hT_d[0][:, 0:32], hT_e[final_par][:, 0:32]).then_inc(s_cast, 1)
    c_cast.add(1)
    cast_init_d = c_cast.v
    h0_qcnt = {}
    for j in range(1, 8):
        rdests = [None] * 8
        rdests[j] = (0, j)
        q = DEC_Q[j]
        nc.gpsimd.remote_dma_broadcast(
            out_ap=hT_d[0][:, j * 32:(j + 1) * 32], in_ap=hT_d[0][:, 0:32],
            remote_sem=s_recv[(j, 0)], local_sem=s_loc[0], rdests=rdests,
            queue_num=q,
        ).then_inc(s_prep, 1)
        c_prep.add(1)
        recv_cnt[(j, 0)] += 2
        h0_qcnt[q] = h0_qcnt.get(q, 0) + 1
    loc_cnt[0] += 112
    nc.gpsimd.wait_ge(s_prep, c_prep.v)
    nc.gpsimd.wait_ge(s_cast, cast_init_d)
    for q, cq in sorted(h0_qcnt.items()):
        nc.gpsimd.trigger_dma(count=cq, queue_num=q)

    nc.all_engine_barrier()

    # ============ phase P3 (fallback, only when not interleaved) ============
    if p3_sched is None:
        assert dembT_relu_done.v > 0
        nc.tensor.wait_ge(s_relu, dembT_relu_done.v)
        ig_precompute(ntk // 512, dec_w1T, 8, dembT_h, ntk, 8 * ntk, ig_d_h, n_t * 128, False, None)

    # ============ phase P4: ctx gates + fold into ig_d ============
    for j in range(1, 8):
        nc.tensor.wait_ge(s_recv[(j, 0)], recv_cnt[(j, 0)])
    nc.tensor.wait_ge(s_cast, cast_init_d)
    evac_base4 = c_evac.v
    for m in range(4):
        bank = pb[m]
        nc.tensor.wait_ge(s_evac, c_evac.v)  # banks were used by P3 tail
        ins = None
        for k in range(8):
            ins = nc.tensor.matmul(
                bank[:, 0:32],
                dec_w2T[:, k * 512 + m * 128: k * 512 + (m + 1) * 128],
                hT_d[0][:, k * 32:(k + 1) * 32],
                start=(k == 0), stop=(k == 7),
            )
        ins.then_inc(s_pe, 1); c_pe.add(1)
        nc.scalar.wait_ge(s_pe, c_pe.v)
        nc.scalar.activation(ctx_g[:, m * 32:(m + 1) * 32], bank[:, 0:32],
                             AF.Identity, bias=dec_b[:, m:m + 1]).then_inc(s_evac, 1)
        c_evac.add(1)
    nc.vector.wait_ge(s_evac, c_evac.v)
    ctx_rep = AP(ctx_g_h, 0, [[128, 128], [0, n_t], [1, 128]])
    igd_3d = AP(ig_d_h, 0, [[n_t * 128, 128], [128, n_t], [1, 128]])
    nc.vector.tensor_add(igd_3d, igd_3d, ctx_rep).then_inc(s_add, 1)
    c_add.add(1)

    nc.all_engine_barrier()

    # ============ phase P5: decoder recurrence ============
    emit_recurrence(n_t, 8, 7, hT_d, c_d, dec_uT, ig_d, outT, None, DEC_Q)

    # ============ output ============
    nc.sync.wait_ge(s_cast, c_cast.v)
    nc.sync.dma_start(out=out_d[:], in_=outT[:, :]).then_inc(s_dma, 16)
    c_dma.add(16)
    nc.sync.wait_ge(s_dma, c_dma.v)

    nc.compile()
    return nc


# ---------------------------------------------------------------------------
# host-side data prep
# ---------------------------------------------------------------------------

def _wrap_idx(idx_flat):
    n = idx_flat.shape[0]
    w = idx_flat.astype(np.int16).reshape(n // 16, 16).T
    return np.ascontiguousarray(np.tile(w, (8, 1)))


def _wT_sbuf(WT, chunk_rows, gate_cols):
    """WT: [Din, 4H] (= W.T); -> [128, nk*512] bf16 SBUF layout."""
    nk = len(chunk_rows)
    out = np.empty((128, nk * 512), NP_BF16)
    for j, r0 in enumerate(chunk_rows):
        out[:, j * 512:(j + 1) * 512] = WT[r0:r0 + 128][:, gate_cols].astype(NP_BF16)
    return out


def _gate_cols(hs, H):
    cols = np.empty(512, np.int64)
    for m in range(4):
        g = _GATE_BLOCK[m]
        cols[m * 128:(m + 1) * 128] = np.arange(hs, hs + 128) + g * H
    return cols


def prepare_in_maps(inputs, slot_map, n_s=S, n_t=T):
    tokens = np.asarray(inputs["tokens"]).astype(np.int64)[:, :n_s]
    trg = np.asarray(inputs["trg_seqs"]).astype(np.int64)[:, :n_t]
    dec_in = np.concatenate([np.full((B, 1), 1, np.int64), trg[:, :-1]], axis=1)

    enc_tab = np.asarray(inputs["enc_emb"]).astype(NP_BF16)
    dec_tab = np.asarray(inputs["dec_emb"]).astype(NP_BF16)

    quadA = sorted(int(x) for x in slot_map[0, :4])
    quadB = sorted(int(x) for x in set(range(8)) - set(quadA))
    is_fwd = {c: (c in quadA) for c in range(8)}
    qrank = {}
    for q in (quadA, quadB):
        for a, c in enumerate(q):
            qrank[c] = a
    enc_rows = {c: (qrank[c] * 128 if is_fwd[c] else 512 + qrank[c] * 128)
                for c in range(8)}

    WihT = {True: np.asarray(inputs["enc_Wih_f"]).T, False: np.asarray(inputs["enc_Wih_b"]).T}
    WhhT = {True: np.asarray(inputs["enc_Whh_f"]).T, False: np.asarray(inputs["enc_Whh_b"]).T}
    enc_bias = {True: np.asarray(inputs["enc_b_f"]), False: np.asarray(inputs["enc_b_b"])}
    W1T = np.asarray(inputs["dec_Wih"])[:, :HD].T
    W2T = np.asarray(inputs["dec_Wih"])[:, HD:].T
    UT = np.asarray(inputs["dec_Whh"]).T
    db = np.asarray(inputs["dec_b"])

    dec_idx_w = _wrap_idx(dec_in.T.reshape(-1))

    in_maps = []
    for r in range(8):
        fwd = is_fwd[r]
        hs = qrank[r] * 128
        gcols_e = _gate_cols(hs, E)
        enc_wT = _wT_sbuf(WihT[fwd], [0, 128, 256, 384], gcols_e)
        chunk_rows = [qrank[int(slot_map[r, j])] * 128 for j in range(4)]
        enc_uT = _wT_sbuf(WhhT[fwd], chunk_rows, gcols_e)
        eb = np.empty((128, 4), np.float32)
        for m in range(4):
            g = _GATE_BLOCK[m]
            eb[:, m] = enc_bias[fwd][g * E + hs: g * E + hs + 128]
        tk = tokens if fwd else tokens[:, ::-1]
        enc_idx_w = _wrap_idx(tk.T.reshape(-1))

        hs_d = r * 128
        gcols_d = _gate_cols(hs_d, HD)
        dec_w1T = _wT_sbuf(W1T, [128 * k for k in range(8)], gcols_d)
        w2_rows = [enc_rows[int(slot_map[r, j])] for j in range(8)]
        dec_w2T = _wT_sbuf(W2T, w2_rows, gcols_d)
        u_rows = [int(slot_map[r, j]) * 128 for j in range(8)]
        dec_uT = _wT_sbuf(UT, u_rows, gcols_d)
        dbv = np.empty((128, 4), np.float32)
        for m in range(4):
            g = _GATE_BLOCK[m]
            dbv[:, m] = db[g * HD + hs_d: g * HD + hs_d + 128]

        in_maps.append(dict(
            enc_tab=enc_tab, dec_tab=dec_tab,
            enc_idx=enc_idx_w, dec_idx=dec_idx_w,
            enc_wT=enc_wT, enc_uT=enc_uT, enc_b=eb,
            dec_w1T=dec_w1T, dec_w2T=dec_w2T, dec_uT=dec_uT, dec_b=dbv,
        ))
    return in_maps


def assemble_output(results, n_t=T):
    decoded = np.empty((B, n_t, HD), np.float32)
    for r in range(8):
        o = np.asarray(results[r]["out"])  # [128, n_t*32]
        decoded[:, :, r * 128:(r + 1) * 128] = (
            o.reshape(128, n_t, 32).transpose(2, 1, 0)
        )
    return decoded


_KERNEL_CACHE = {}


def _get_kernel(n_s, n_t):
    key = (n_s, n_t)
    if key not in _KERNEL_CACHE:
        _KERNEL_CACHE[key] = build_kernel(n_s, n_t)
    return _KERNEL_CACHE[key]


def run(inputs, n_s=S, n_t=T, trace=False):
    slot_map = get_slot_map()
    nc = _get_kernel(n_s, n_t)
    in_maps = prepare_in_maps(inputs, slot_map, n_s, n_t)
    res = run_bass_kernel_spmd(nc, in_maps, core_ids=list(range(NC)), trace=trace)
    return assemble_output(res.results, n_t), res


def kernel(**inputs) -> np.ndarray:
    out, _ = run(inputs)
    return out

